# revision 53
# baseline (speedup 1.0000x reference)
"""Trainium2 Bass kernel for nn_LocationAwareMSAGAT_Net.

Strategy: data-parallel over batch B=8 across the 8 NeuronCores (one batch
element per core); all parameters replicated.  Per core:

  phase A: multi-scale dilated conv (as shifted matmuls over full N, bf16)
           + BN fold + SiLU (ScalarE, conv bias folded into activation bias)
  phase B: bottleneck (alpha folded into W_low; accumulated in PSUM over
           scales) -> W_high -> +residual -> LayerNorm1 -> transpose (PE)
  phase C: GAT projections: one matmul computes Wh for all heads plus
           src/dst attention logits (gat_W@a_src / gat_W@a_dst appended as
           extra columns)
  phase D: attention, computed transposed (P^T[m,q] tiles):
           ptl = leaky(srcb[q] + dst[m] + maskbias) in ONE custom DVE op
           (LEAKY_SCORE_ANT: max(y, 0.2y), y = in0 + s0 + in1)
           exp on ScalarE (two [128, 4096] batches per head)
           hp^T = [Wh_h | ones]^T @ P^T accumulated in PSUM over m-chunks
           (ones column yields softmax denominators)
           PE-transpose back, divide rows by denominator
  phase E: LayerNorm2 -> DMA out

Everything on the PE is bf16 with fp32 PSUM accumulation.  PSUM->SBUF
staging copies ride on GpSimd (otherwise idle) to keep DVE free for the
attention elementwise work.
"""

import os
import numpy as np
import ml_dtypes
from contextlib import ExitStack

import concourse.bass as bass
import concourse.tile as tile
from concourse import bacc, mybir
from concourse.bass_utils import run_bass_kernel_spmd
from concourse.masks import make_identity

BF = mybir.dt.bfloat16
FP8 = mybir.dt.float8e4
F16 = mybir.dt.float16
F32 = mybir.dt.float32
EPS = 1e-5
NEG = -1e9
LEAKY_SLOPE = 0.2
WSCALE = 32.0           # conv weights pre-scaled into e4m3 normal range

B, N, H = 8, 1024, 256
S, K, HEADS = 4, 3, 4
D = H // HEADS          # 64
NCH = N // 128          # 8 chunks of 128
CCH = H // 128          # 2 channel chunks
BOT = 8                 # bottleneck dim

_CACHED = {}


def _register_leaky_op():
    """Custom DVE op: out = leaky_relu(in0 + s0 + in1, slope=imm2).

    One DVE pass (1 elem/cycle/lane) replacing the gpsimd-add + DVE-add +
    scalar_tensor_tensor chain.  Registered at runtime into dve_ops.OPS."""
    import concourse.dve_ops as dops
    from concourse.dve_spec import Spec, Src0, Src1, C0, C2, maxx, lower, \
        _has_src1
    from concourse.dve_uop import DveOpSpec

    name = "LEAKY_SCORE_ANT"
    for op in dops.OPS:
        if op.name == name:
            return op

    _y = (Src0 + C0) + Src1

    def _ref(in0, in1, s0, s1, imm2):
        y = (in0.astype(np.float32) + s0) + in1
        return np.maximum(y, y * imm2)

    spec = Spec(body=maxx(_y, _y * C2), reference=_ref)
    row = max(dops._SUB_OPCODE_FOR_NAME.values()) + 1
    dops._SUB_OPCODE_FOR_NAME[name] = row
    uops = lower(spec, ver="v3")
    sha = DveOpSpec(name=name, opcode=row, uops=uops,
                    rd1_en=_has_src1(spec)).sha("v3")
    op = dops.DveOp(name, spec, subdim=False, uops_sha={"v3": sha})
    dops.OPS.append(op)
    dops.CUSTOM_DVE_SPECS[name] = spec
    return op


LEAKY_OP = _register_leaky_op()


def _patch_act_tables():
    """Drop exp_and_others so Exp resolves to natural_log_exp_and_others:
    Ln/Exp/Prelu/Silu then need only 2 table sets total and the rstd2
    (LayerNorm2) tail pays no table switches."""
    import concourse.hw_specs as hw_specs
    import concourse.bacc as bacc_mod
    orig = hw_specs.get_activation_tables
    if getattr(hw_specs, "_leaky_patched", False):
        return

    def _gat(arch):
        t = dict(orig(arch))
        if "exp_and_others" in t:
            # keep the slot (dict order = act_func_set_id) but make it
            # unselectable so Exp lands in natural_log_exp_and_others
            t["exp_and_others"] = set()
        return t

    hw_specs.get_activation_tables = _gat
    bacc_mod.get_activation_tables = _gat
    hw_specs._leaky_patched = True


_patch_act_tables()


def _patch_ldw_opt():
    """Enable walrus's LDWEIGHTS optimization (off by default in bass)."""
    import concourse.bass_utils as bu
    if getattr(bu, "_ldw_patched", False):
        return
    orig = bu.run_command

    def _rc(argv, **kw):
        argv = ["--enable-ldw-opt=true" if a == "--enable-ldw-opt=false" else a
                for a in argv]
        return orig(argv, **kw)

    bu.run_command = _rc
    bu._ldw_patched = True


if os.environ.get("BASS_LDW_OPT", "0") == "1":
    _patch_ldw_opt()


def _build(trivial: dict) -> bass.Bass:
    nc = bacc.Bacc("TRN2", target_bir_lowering=False, debug=False,
                   num_devices=B)

    # All inputs are staged host-side in their final [128, ...] SBUF layout
    # so every DMA is one contiguous run per partition (cheap descriptors,
    # fast issue).
    xt_d = nc.declare_dram_parameter("xt", [128, CCH, N], FP8, isOutput=False)
    xres_d = nc.declare_dram_parameter("xres", [128, NCH, H], F16, isOutput=False)
    wt_d = nc.declare_dram_parameter("wt", [128, S * K * CCH, H], FP8, isOutput=False)
    bconv_d = nc.declare_dram_parameter("bconv", [128, S * CCH], F32, isOutput=False)
    wlow_d = nc.declare_dram_parameter("wlow", [128, S * CCH, BOT], BF, isOutput=False)
    whigh_d = nc.declare_dram_parameter("whigh", [BOT, H], BF, isOutput=False)
    g_d = nc.declare_dram_parameter("gmat", [128, CCH, H + 2 * HEADS], BF,
                                    isOutput=False)
    mask_d = nc.declare_dram_parameter("maskT", [128, NCH, N], BF, isOutput=False)
    wsr_d = nc.declare_dram_parameter("wsrcrep", [128, HEADS, CCH, 128], BF,
                                      isOutput=False)
    out_d = nc.declare_dram_parameter("out", [N, H], F16, isOutput=True)

    with tile.TileContext(nc) as tc:
        with ExitStack() as ctx:
            _body(ctx, tc, xt_d, xres_d, wt_d, bconv_d, wlow_d, whigh_d, g_d,
                  mask_d, wsr_d, out_d)
    nc.compile()
    return nc


def _body(ctx, tc, xt_d, xres_d, wt_d, bconv_d, wlow_d, whigh_d, g_d,
          mask_d, wsr_d, out_d):
    nc = tc.nc
    consts = ctx.enter_context(tc.tile_pool(name="consts", bufs=1))
    work = ctx.enter_context(tc.tile_pool(name="work", bufs=3))
    statp = ctx.enter_context(tc.tile_pool(name="stats", bufs=4))
    outp = ctx.enter_context(tc.tile_pool(name="outp", bufs=8))
    ptp = ctx.enter_context(tc.tile_pool(name="ptp", bufs=2))
    ptlp = ctx.enter_context(tc.tile_pool(name="ptlp", bufs=2))

    # ---------------- constants / inputs into SBUF ----------------
    xpad = consts.tile([128, CCH, N + 16], FP8, tag="xpad")
    nc.vector.memset(xpad[:, :, 0:8], 0.0)
    nc.vector.memset(xpad[:, :, N + 8:N + 16], 0.0)
    nc.sync.dma_start(out=xpad[:, :, 8:8 + N], in_=xt_d[:])

    wt_sb = consts.tile([128, S * K * CCH, H], FP8, tag="wt")
    # split so conv on scales 0-1 can start before scales 2-3 land
    nc.scalar.dma_start(out=wt_sb[:, 0:S * K * CCH // 2, :],
                        in_=wt_d[:, 0:S * K * CCH // 2, :])
    nc.scalar.dma_start(out=wt_sb[:, S * K * CCH // 2:, :],
                        in_=wt_d[:, S * K * CCH // 2:, :])

    bconv_sb = consts.tile([128, S * CCH], F32, tag="bconv")
    nc.sync.dma_start(out=bconv_sb[:], in_=bconv_d[:])

    wlow_sb = consts.tile([128, S * CCH, BOT], BF, tag="wlow")
    nc.sync.dma_start(out=wlow_sb[:], in_=wlow_d[:])

    whigh_sb = consts.tile([BOT, H], BF, tag="whigh")
    nc.sync.dma_start(out=whigh_sb[:], in_=whigh_d[:])

    xres_sb = consts.tile([128, NCH, H], F16, tag="xres")
    nc.sync.dma_start(out=xres_sb[:], in_=xres_d[:])

    g_sb = consts.tile([128, CCH, H + 2 * HEADS], BF, tag="gmat")
    nc.scalar.dma_start(out=g_sb[:], in_=g_d[:])

    wsr_sb = consts.tile([128, HEADS, CCH, 128], BF, tag="wsr")
    nc.sync.dma_start(out=wsr_sb[:], in_=wsr_d[:])

    # mask (2MB, needed only in phase D) is DMA'd from inside phase A so it
    # doesn't compete with xt/wt/xres for startup bandwidth
    mask_sb = consts.tile([128, NCH, N], BF, tag="mask")

    ident_bf = consts.tile([128, 128], BF, tag="idbf")
    make_identity(nc, ident_bf[:])
    ident_f32 = consts.tile([128, 128], F32, tag="idf32")
    make_identity(nc, ident_f32[:])
    eps_sb = consts.tile([128, 1], F32, tag="eps")
    nc.vector.memset(eps_sb[:], EPS)
    zero_sb = consts.tile([128, 1], F32, tag="zero")
    nc.vector.memset(zero_sb[:], 0.0)

    # persistent intermediates
    fused_sb = consts.tile([128, S, CCH, N], BF, tag="fused")
    lowT_sb = consts.tile([BOT, N], BF, tag="lowT")
    h_all = consts.tile([128, NCH, H], F32, tag="h_all")
    mv1 = consts.tile([128, NCH, 2], F32, tag="mv1")
    rstd1 = consts.tile([128, NCH], F32, tag="rstd1")
    hT_sb = consts.tile([128, CCH, N], BF, tag="hT")
    wh_all = consts.tile([128, NCH, HEADS * (D + 1)], BF, tag="wh")
    nc.vector.memset(
        wh_all[:].rearrange("p j (h x) -> p j h x", x=D + 1)[:, :, :, D], 1.0)
    sd_sb = consts.tile([128, NCH, 2 * HEADS], F32, tag="sd")
    srcb_sb = consts.tile([128, HEADS, N], BF, tag="srcb")
    hp_all = consts.tile([128, NCH, H], F32, tag="hp")
    mv2 = consts.tile([128, NCH, 2], F32, tag="mv2")
    rstd2 = consts.tile([128, NCH], F32, tag="rstd2")

    # ---------------- phase A: conv + silu ----------------
    ctxB = ExitStack()
    psA = ctxB.enter_context(tc.tile_pool(name="psB", bufs=2, space="PSUM"))
    psTr = ctxB.enter_context(tc.tile_pool(name="psTrB", bufs=2, space="PSUM"))
    ctxA = ExitStack()
    convp = ctxA.enter_context(tc.tile_pool(name="convp", bufs=3, space="PSUM"))
    lowp = ctxA.enter_context(tc.tile_pool(name="lowp", bufs=1, space="PSUM"))
    WINV = 1.0 / WSCALE
    for nch in range(2):               # halves of N, 512 wide
        if nch == 1:
            nc.scalar.dma_start(out=mask_sb[:], in_=mask_d[:])
        for i in range(S):
            for cout in range(CCH):
                ps = convp.tile([128, 512], F32, tag="conv")
                dil = 2 ** i
                for k in range(K):
                    # fp8 DoubleRow: one matmul contracts both channel
                    # chunks (lhsT [128,2,128], rhs [128,2,512])
                    sh = (k - 1) * dil
                    t0 = (i * K + k) * CCH
                    a = 8 + sh + nch * 512
                    nc.tensor.matmul(
                        ps[:],
                        lhsT=wt_sb[:, t0:t0 + 2, cout * 128:(cout + 1) * 128],
                        rhs=xpad[:, :, a:a + 512],
                        start=(k == 0), stop=(k == K - 1),
                        perf_mode=mybir.MatmulPerfMode.DoubleRow)
                dst = fused_sb[:, i, cout, nch * 512:nch * 512 + 512]
                bias_ap = bconv_sb[:, i * CCH + cout:i * CCH + cout + 1]
                if os.environ.get("BASS_SIM_COMPAT", "0") == "1":
                    # CoreSim has no Silu: sigmoid + fused (t+b)*sig on DVE
                    t_ = work.tile([128, 512], F32, tag="tconv")
                    nc.scalar.mul(t_[:], ps[:], WINV)
                    sg = work.tile([128, 512], F32, tag="sg")
                    nc.scalar.activation(
                        out=sg[:], in_=t_[:],
                        func=mybir.ActivationFunctionType.Sigmoid,
                        bias=bias_ap, scale=1.0)
                    nc.vector.scalar_tensor_tensor(
                        out=dst, in0=t_[:], scalar=bias_ap, in1=sg[:],
                        op0=mybir.AluOpType.add, op1=mybir.AluOpType.mult)
                else:
                    nc.scalar.activation(
                        out=dst, in_=ps[:],
                        func=mybir.ActivationFunctionType.Silu,
                        bias=bias_ap, scale=WINV)

        # phase A2 for this half: lowT = sum_i (a_i W_low)^T @ silu_i
        lps = lowp.tile([BOT, 512], F32, tag="low")
        first = True
        for i in range(S):
            for c in range(CCH):
                nc.tensor.matmul(
                    lps[:],
                    lhsT=wlow_sb[:, i * CCH + c, :],
                    rhs=fused_sb[:, i, c, nch * 512:nch * 512 + 512],
                    start=first, stop=(i == S - 1 and c == CCH - 1))
                first = False
        nc.vector.tensor_copy(out=lowT_sb[:, nch * 512:nch * 512 + 512],
                              in_=lps[:])

        # phase B for this half: high + residual + LN1 stats
        for q in range(nch * 4, nch * 4 + 4):
            hps = psA.tile([128, H], F32, tag="high")
            nc.tensor.matmul(hps[:], lhsT=lowT_sb[:, q * 128:(q + 1) * 128],
                             rhs=whigh_sb[:], start=True, stop=True)
            nc.vector.tensor_add(h_all[:, q, :], hps[:], xres_sb[:, q, :])
            st = statp.tile([128, 6], F32, tag="bn1")
            nc.vector.bn_stats(out=st[:], in_=h_all[:, q, :])
            nc.vector.bn_aggr(out=mv1[:, q, :], in_=st[:])
    ctxA.close()

    # rstd1 = exp(-0.5 * ln(var + eps))  (one table set: natural_log+exp)
    nc.scalar.activation(out=rstd1[:], in_=mv1[:, :, 1],
                         func=mybir.ActivationFunctionType.Ln, bias=eps_sb[:],
                         scale=1.0)
    nc.scalar.activation(out=rstd1[:], in_=rstd1[:],
                         func=mybir.ActivationFunctionType.Exp, bias=zero_sb[:],
                         scale=-0.5)

    for q in range(NCH):
        hn = work.tile([128, H], BF, tag="hn")
        nc.vector.tensor_scalar(
            out=hn[:], in0=h_all[:, q, :],
            scalar1=mv1[:, q, 0:1], scalar2=rstd1[:, q:q + 1],
            op0=mybir.AluOpType.subtract, op1=mybir.AluOpType.mult)
        for c in range(CCH):
            tp = psTr.tile([128, 128], BF, tag="trh")
            nc.tensor.transpose(out=tp[:], in_=hn[:, c * 128:(c + 1) * 128],
                                identity=ident_bf[:])
            nc.scalar.copy(out=hT_sb[:, c, q * 128:(q + 1) * 128],
                           in_=tp[:])

    ctxB.close()
    # ---------------- phase C: GAT projections ----------------
    ctxC = ExitStack()
    psA = ctxC.enter_context(tc.tile_pool(name="psC", bufs=2, space="PSUM"))
    psTr = ctxC.enter_context(tc.tile_pool(name="psTrC", bufs=2, space="PSUM"))
    for j in range(NCH):
        gps = psA.tile([128, H + 2 * HEADS], F32, tag="gat")
        for c in range(CCH):
            nc.tensor.matmul(gps[:], lhsT=hT_sb[:, c, j * 128:(j + 1) * 128],
                             rhs=g_sb[:, c, :], start=(c == 0),
                             stop=(c == CCH - 1))
        whj = wh_all[:, j, :].rearrange("p (h x) -> p h x", x=D + 1)
        nc.scalar.copy(
            out=whj[:, :, 0:D],
            in_=gps[:, 0:H].rearrange("p (h x) -> p h x", x=D))
        nc.vector.tensor_copy(out=sd_sb[:, j, :], in_=gps[:, H:H + 2 * HEADS])

    # src_bcast[h][p, q] = src_h[q] for all p, via replicated-column matmul
    for h in range(HEADS):
        for half in range(2):
            sps = psTr.tile([128, 512], F32, tag="sbc")
            for c in range(CCH):
                nc.tensor.matmul(
                    sps[:], lhsT=wsr_sb[:, h, c, :],
                    rhs=hT_sb[:, c, half * 512:half * 512 + 512],
                    start=(c == 0), stop=(c == CCH - 1))
            nc.scalar.copy(out=srcb_sb[:, h, half * 512:half * 512 + 512],
                           in_=sps[:])

    ctxC.close()
    # ---------------- phase D: attention ----------------
    ctxD = ExitStack()
    attp = ctxD.enter_context(tc.tile_pool(name="attp", bufs=4, space="PSUM"))
    psTr = ctxD.enter_context(tc.tile_pool(name="psTrD", bufs=3, space="PSUM"))
    J_SCALAR = ()       # gpsimd add + ScalarE Prelu (empty: all chunks on DVE)
    for h in range(HEADS):
        ptl = ptlp.tile([128, NCH, N], BF, tag="ptl")
        pt = ptp.tile([128, NCH, N], BF, tag="pt")
        for j in J_SCALAR:
            cs = work.tile([128, N], BF, tag="cs")
            nc.gpsimd.tensor_tensor(out=cs[:], in0=srcb_sb[:, h, :],
                                    in1=mask_sb[:, j, :],
                                    op=mybir.AluOpType.add)
            nc.scalar.activation(
                out=ptl[:, j, :], in_=cs[:],
                func=mybir.ActivationFunctionType.Prelu,
                bias=sd_sb[:, j, HEADS + h:HEADS + h + 1],
                scale=1.0, alpha=LEAKY_SLOPE)
        for j in range(NCH):
            if j in J_SCALAR:
                continue
            # ptl = leaky(srcb[q] + dst[m] + mask) in one DVE pass
            nc.vector._custom_dve(
                LEAKY_OP, out=ptl[:, j, :], in0=srcb_sb[:, h, :],
                in1=mask_sb[:, j, :],
                s0=sd_sb[:, j, HEADS + h:HEADS + h + 1], imm2=LEAKY_SLOPE)
        # last head: quarter-split exp so downstream PE/DVE start sooner
        nexp = 4 if h == HEADS - 1 else 2
        step = NCH // nexp
        for part in range(nexp):
            nc.scalar.activation(
                out=pt[:, part * step:(part + 1) * step, :],
                in_=ptl[:, part * step:(part + 1) * step, :],
                func=mybir.ActivationFunctionType.Exp,
                bias=zero_sb[:], scale=1.0)

        hp0 = attp.tile([D + 1, 512], F32, tag="hpT")
        hp1 = attp.tile([D + 1, 512], F32, tag="hpT")
        for j in range(NCH):
            for half, hps_ in ((0, hp0), (1, hp1)):
                nc.tensor.matmul(
                    hps_[:],
                    lhsT=wh_all[:, j, h * (D + 1):(h + 1) * (D + 1)],
                    rhs=pt[:, j, half * 512:half * 512 + 512],
                    start=(j == 0), stop=(j == NCH - 1))
        hpt = work.tile([D + 1, N], F32, tag="hpt")
        nc.scalar.copy(out=hpt[:, 0:512], in_=hp0[:])
        nc.scalar.copy(out=hpt[:, 512:N], in_=hp1[:])
        for q in range(NCH):
            tq = psTr.tile([128, D + 1], F32, tag="trq")
            nc.tensor.transpose(out=tq[:], in_=hpt[:, q * 128:(q + 1) * 128],
                                identity=ident_f32[0:D + 1, 0:D + 1])
            rd = statp.tile([128, 1], F32, tag="rd")
            nc.vector.reciprocal(out=rd[:], in_=tq[:, D:D + 1])
            nc.vector.tensor_scalar_mul(
                out=hp_all[:, q, h * D:(h + 1) * D],
                in0=tq[:, 0:D], scalar1=rd[:])
            if h == HEADS - 1:
                # interleave LN2 stats with the last head's normalize so the
                # in-order DVE queue doesn't serialize them at the end
                st = statp.tile([128, 6], F32, tag="bn2")
                nc.vector.bn_stats(out=st[:], in_=hp_all[:, q, :])
                nc.vector.bn_aggr(out=mv2[:, q, :], in_=st[:])

    ctxD.close()
    # ---------------- phase E: ln2 + out ----------------
    nc.scalar.activation(out=rstd2[:], in_=mv2[:, :, 1],
                         func=mybir.ActivationFunctionType.Ln, bias=eps_sb[:],
                         scale=1.0)
    nc.scalar.activation(out=rstd2[:], in_=rstd2[:],
                         func=mybir.ActivationFunctionType.Exp, bias=zero_sb[:],
                         scale=-0.5)
    for q in range(NCH):
        ot = outp.tile([128, H], F16, tag="out")
        nc.vector.tensor_scalar(
            out=ot[:], in0=hp_all[:, q, :],
            scalar1=mv2[:, q, 0:1], scalar2=rstd2[:, q:q + 1],
            op0=mybir.AluOpType.subtract, op1=mybir.AluOpType.mult)
        eng = nc.sync if q % 2 == 0 else nc.scalar
        eng.dma_start(out=out_d[q * 128:(q + 1) * 128, :], in_=ot[:])


def _prep(inputs):
    """Host-side parameter folding. Returns per-core input maps."""
    bf16 = ml_dtypes.bfloat16
    f = lambda a: np.ascontiguousarray(np.asarray(a, np.float32))

    x = f(inputs["x"])
    adj = np.asarray(inputs["adj"])
    conv_w = f(inputs["conv_w"]); conv_b = f(inputs["conv_b"])
    bn_g = f(inputs["bn_g"]); bn_b = f(inputs["bn_b"])
    fw = f(inputs["fusion_weight"])
    W_low = f(inputs["W_low"]); b_low = f(inputs["b_low"])
    W_high = f(inputs["W_high"]); b_high = f(inputs["b_high"])
    ln1_g = f(inputs["ln1_g"]); ln1_b = f(inputs["ln1_b"])
    gat_W = f(inputs["gat_W"])
    a_src = f(inputs["a_src"]); a_dst = f(inputs["a_dst"])
    ln2_g = f(inputs["ln2_g"]); ln2_b = f(inputs["ln2_b"])

    trivial = dict(
        b_low=np.allclose(b_low, 0), b_high=np.allclose(b_high, 0),
        ln1=np.allclose(ln1_g, 1) and np.allclose(ln1_b, 0),
        ln2=np.allclose(ln2_g, 1) and np.allclose(ln2_b, 0))
    if not all(trivial.values()):
        raise NotImplementedError(f"non-trivial affine params: {trivial}")

    alpha = np.exp(fw - fw.max()); alpha /= alpha.sum()
    gprime = bn_g / np.float32(np.sqrt(1.0 + EPS))          # [S,H]
    bconv = conv_b * gprime + bn_b                           # [S,H]
    # Wt[i,k,cin,cout] = conv_w[i,cout,cin,k]*gprime[i,cout], pre-scaled by
    # WSCALE so e4m3 keeps ~3 mantissa bits (unscaled in the SiLU activation)
    Wt = np.transpose(conv_w, (0, 3, 2, 1)) * gprime[:, None, None, :] * WSCALE
    # [S,K,cin,H] -> [S*K*CCH,128,H] -> partition-major [128,S*K*CCH,H]
    Wt = Wt.reshape(S, K, CCH, 128, H).reshape(S * K * CCH, 128, H)
    Wt = np.ascontiguousarray(Wt.transpose(1, 0, 2))
    # bconv laid out [128, S*CCH]: column i*CCH+c holds channels c*128..c*128+127
    bconv_t = bconv.reshape(S, CCH, 128).transpose(2, 0, 1).reshape(128, S * CCH)

    WlowA = (alpha[:, None, None] * W_low[None]).reshape(S, CCH, 128, BOT)
    WlowA = np.ascontiguousarray(
        WlowA.reshape(S * CCH, 128, BOT).transpose(1, 0, 2))

    G = np.zeros((H, H + 2 * HEADS), np.float32)
    for h in range(HEADS):
        G[:, h * D:(h + 1) * D] = gat_W[h]
        G[:, H + h] = gat_W[h] @ a_src[h]
        G[:, H + HEADS + h] = gat_W[h] @ a_dst[h]
    Gr = np.ascontiguousarray(
        G.reshape(CCH, 128, H + 2 * HEADS).transpose(1, 0, 2))

    maskT = np.where(adj.T > 0, np.float32(0.0), np.float32(NEG))
    maskTr = np.ascontiguousarray(maskT.reshape(NCH, 128, N).transpose(1, 0, 2))

    # wsrcrep[:, h, c, j] = (gat_W[h] @ a_src[h])[c*128 + :]  (all 128 cols equal)
    wsrc = np.stack([gat_W[h] @ a_src[h] for h in range(HEADS)])  # [HEADS, H]
    wsrcrep = np.repeat(
        wsrc.reshape(HEADS, CCH, 128, 1), 128, axis=3).astype(np.float32)
    wsrcrep = np.ascontiguousarray(wsrcrep.transpose(2, 0, 1, 3))

    fp8 = ml_dtypes.float8_e4m3
    shared = {
        "wt": Wt.astype(fp8),
        "bconv": np.ascontiguousarray(bconv_t),
        "wlow": WlowA.astype(bf16),
        "whigh": W_high.astype(bf16),
        "gmat": Gr.astype(bf16),
        "maskT": maskTr.astype(bf16),
        "wsrcrep": wsrcrep.astype(bf16),
    }
    in_maps = []
    for b in range(B):
        xt = np.ascontiguousarray(x[b].T)                    # [H, N]
        m = dict(shared)
        m["xt"] = np.ascontiguousarray(
            xt.reshape(CCH, 128, N).transpose(1, 0, 2)).astype(fp8)
        m["xres"] = np.ascontiguousarray(
            x[b].reshape(NCH, 128, H).transpose(1, 0, 2)).astype(np.float16)
        in_maps.append(m)
    return in_maps, trivial


def kernel(**inputs) -> np.ndarray:
    in_maps, trivial = _prep(inputs)
    key = "k"
    if key not in _CACHED:
        _CACHED[key] = _build(trivial)
    nc = _CACHED[key]
    res = run_bass_kernel_spmd(nc, in_maps, list(range(B)))
    out = np.stack([res.results[i]["out"] for i in range(B)], axis=0)
    return out.astype(np.float32)


if __name__ == "__main__":
    import reference
    inputs = {k: np.asarray(v) for k, v in reference.setup_inputs().items()}
    got = kernel(**inputs)
    print("kernel output", got.shape, got.dtype)


# revision 54
# speedup vs baseline: 1.0115x; 1.0115x over previous
"""Trainium2 Bass kernel for nn_LocationAwareMSAGAT_Net.

Strategy: data-parallel over batch B=8 across the 8 NeuronCores (one batch
element per core); all parameters replicated.  Per core:

  phase A: multi-scale dilated conv (as shifted matmuls over full N, bf16)
           + BN fold + SiLU (ScalarE, conv bias folded into activation bias)
  phase B: bottleneck (alpha folded into W_low; accumulated in PSUM over
           scales) -> W_high -> +residual -> LayerNorm1 -> transpose (PE)
  phase C: GAT projections: one matmul computes Wh for all heads plus
           src/dst attention logits (gat_W@a_src / gat_W@a_dst appended as
           extra columns)
  phase D: attention, computed transposed (P^T[m,q] tiles):
           ptl = leaky(srcb[q] + dst[m] + maskbias) in ONE custom DVE op
           (LEAKY_SCORE_ANT: max(y, 0.2y), y = in0 + s0 + in1)
           exp on ScalarE (two [128, 4096] batches per head)
           hp^T = [Wh_h | ones]^T @ P^T accumulated in PSUM over m-chunks
           (ones column yields softmax denominators)
           PE-transpose back, divide rows by denominator
  phase E: LayerNorm2 -> DMA out

Everything on the PE is bf16 with fp32 PSUM accumulation.  PSUM->SBUF
staging copies ride on GpSimd (otherwise idle) to keep DVE free for the
attention elementwise work.
"""

import os
import numpy as np
import ml_dtypes
from contextlib import ExitStack

import concourse.bass as bass
import concourse.tile as tile
from concourse import bacc, mybir
from concourse.bass_utils import run_bass_kernel_spmd
from concourse.masks import make_identity

BF = mybir.dt.bfloat16
FP8 = mybir.dt.float8e4
F16 = mybir.dt.float16
F32 = mybir.dt.float32
EPS = 1e-5
NEG = -1e9
LEAKY_SLOPE = 0.2
WSCALE = 32.0           # conv weights pre-scaled into e4m3 normal range

B, N, H = 8, 1024, 256
S, K, HEADS = 4, 3, 4
D = H // HEADS          # 64
NCH = N // 128          # 8 chunks of 128
CCH = H // 128          # 2 channel chunks
BOT = 8                 # bottleneck dim

_CACHED = {}


def _register_leaky_op():
    """Custom DVE op: out = leaky_relu(in0 + s0 + in1, slope=imm2).

    One DVE pass (1 elem/cycle/lane) replacing the gpsimd-add + DVE-add +
    scalar_tensor_tensor chain.  Registered at runtime into dve_ops.OPS."""
    import concourse.dve_ops as dops
    from concourse.dve_spec import Spec, Src0, Src1, C0, C2, maxx, lower, \
        _has_src1
    from concourse.dve_uop import DveOpSpec

    name = "LEAKY_SCORE_ANT"
    for op in dops.OPS:
        if op.name == name:
            return op

    _y = (Src0 + C0) + Src1

    def _ref(in0, in1, s0, s1, imm2):
        y = (in0.astype(np.float32) + s0) + in1
        return np.maximum(y, y * imm2)

    spec = Spec(body=maxx(_y, _y * C2), reference=_ref)
    row = max(dops._SUB_OPCODE_FOR_NAME.values()) + 1
    dops._SUB_OPCODE_FOR_NAME[name] = row
    uops = lower(spec, ver="v3")
    sha = DveOpSpec(name=name, opcode=row, uops=uops,
                    rd1_en=_has_src1(spec)).sha("v3")
    op = dops.DveOp(name, spec, subdim=False, uops_sha={"v3": sha})
    dops.OPS.append(op)
    dops.CUSTOM_DVE_SPECS[name] = spec
    return op


LEAKY_OP = _register_leaky_op()


def _patch_act_tables():
    """Drop exp_and_others so Exp resolves to natural_log_exp_and_others:
    Ln/Exp/Prelu/Silu then need only 2 table sets total and the rstd2
    (LayerNorm2) tail pays no table switches."""
    import concourse.hw_specs as hw_specs
    import concourse.bacc as bacc_mod
    orig = hw_specs.get_activation_tables
    if getattr(hw_specs, "_leaky_patched", False):
        return

    def _gat(arch):
        t = dict(orig(arch))
        if "exp_and_others" in t:
            # keep the slot (dict order = act_func_set_id) but make it
            # unselectable so Exp lands in natural_log_exp_and_others
            t["exp_and_others"] = set()
        return t

    hw_specs.get_activation_tables = _gat
    bacc_mod.get_activation_tables = _gat
    hw_specs._leaky_patched = True


_patch_act_tables()


def _patch_ldw_opt():
    """Enable walrus's LDWEIGHTS optimization (off by default in bass)."""
    import concourse.bass_utils as bu
    if getattr(bu, "_ldw_patched", False):
        return
    orig = bu.run_command

    def _rc(argv, **kw):
        argv = ["--enable-ldw-opt=true" if a == "--enable-ldw-opt=false" else a
                for a in argv]
        return orig(argv, **kw)

    bu.run_command = _rc
    bu._ldw_patched = True


if os.environ.get("BASS_LDW_OPT", "0") == "1":
    _patch_ldw_opt()


def _build(trivial: dict) -> bass.Bass:
    nc = bacc.Bacc("TRN2", target_bir_lowering=False, debug=False,
                   num_devices=B)

    # All inputs are staged host-side in their final [128, ...] SBUF layout
    # so every DMA is one contiguous run per partition (cheap descriptors,
    # fast issue).
    xt_d = nc.declare_dram_parameter("xt", [128, CCH, N], FP8, isOutput=False)
    xres_d = nc.declare_dram_parameter("xres", [128, NCH, H], F16, isOutput=False)
    wt_d = nc.declare_dram_parameter("wt", [128, S * K * CCH, H], FP8, isOutput=False)
    bconv_d = nc.declare_dram_parameter("bconv", [128, S * CCH], F32, isOutput=False)
    wlow_d = nc.declare_dram_parameter("wlow", [128, S * CCH, BOT], BF, isOutput=False)
    whigh_d = nc.declare_dram_parameter("whigh", [BOT, H], BF, isOutput=False)
    g_d = nc.declare_dram_parameter("gmat", [128, CCH, H + 2 * HEADS], BF,
                                    isOutput=False)
    mask_d = nc.declare_dram_parameter("maskT", [128, NCH, N], BF, isOutput=False)
    wsr_d = nc.declare_dram_parameter("wsrcrep", [128, HEADS, CCH, 128], BF,
                                      isOutput=False)
    out_d = nc.declare_dram_parameter("out", [N, H], F16, isOutput=True)

    with tile.TileContext(nc) as tc:
        with ExitStack() as ctx:
            _body(ctx, tc, xt_d, xres_d, wt_d, bconv_d, wlow_d, whigh_d, g_d,
                  mask_d, wsr_d, out_d)
    nc.compile()
    return nc


def _body(ctx, tc, xt_d, xres_d, wt_d, bconv_d, wlow_d, whigh_d, g_d,
          mask_d, wsr_d, out_d):
    nc = tc.nc
    consts = ctx.enter_context(tc.tile_pool(name="consts", bufs=1))
    work = ctx.enter_context(tc.tile_pool(name="work", bufs=3))
    statp = ctx.enter_context(tc.tile_pool(name="stats", bufs=4))
    outp = ctx.enter_context(tc.tile_pool(name="outp", bufs=8))
    ptp = ctx.enter_context(tc.tile_pool(name="ptp", bufs=2))
    ptlp = ctx.enter_context(tc.tile_pool(name="ptlp", bufs=2))

    # ---------------- constants / inputs into SBUF ----------------
    xpad = consts.tile([128, CCH, N + 16], FP8, tag="xpad")
    nc.vector.memset(xpad[:, :, 0:8], 0.0)
    nc.vector.memset(xpad[:, :, N + 8:N + 16], 0.0)
    nc.sync.dma_start(out=xpad[:, :, 8:8 + N], in_=xt_d[:])

    wt_sb = consts.tile([128, S * K * CCH, H], FP8, tag="wt")
    # split so conv on scales 0-1 can start before scales 2-3 land
    nc.scalar.dma_start(out=wt_sb[:, 0:S * K * CCH // 2, :],
                        in_=wt_d[:, 0:S * K * CCH // 2, :])
    nc.scalar.dma_start(out=wt_sb[:, S * K * CCH // 2:, :],
                        in_=wt_d[:, S * K * CCH // 2:, :])

    bconv_sb = consts.tile([128, S * CCH], F32, tag="bconv")
    nc.sync.dma_start(out=bconv_sb[:], in_=bconv_d[:])

    wlow_sb = consts.tile([128, S * CCH, BOT], BF, tag="wlow")
    nc.sync.dma_start(out=wlow_sb[:], in_=wlow_d[:])

    whigh_sb = consts.tile([BOT, H], BF, tag="whigh")
    nc.sync.dma_start(out=whigh_sb[:], in_=whigh_d[:])

    xres_sb = consts.tile([128, NCH, H], F16, tag="xres")
    nc.sync.dma_start(out=xres_sb[:], in_=xres_d[:])

    g_sb = consts.tile([128, CCH, H + 2 * HEADS], BF, tag="gmat")
    nc.scalar.dma_start(out=g_sb[:], in_=g_d[:])

    wsr_sb = consts.tile([128, HEADS, CCH, 128], BF, tag="wsr")
    nc.sync.dma_start(out=wsr_sb[:], in_=wsr_d[:])

    # mask (2MB, needed only in phase D) is DMA'd from inside phase A so it
    # doesn't compete with xt/wt/xres for startup bandwidth
    mask_sb = consts.tile([128, NCH, N], BF, tag="mask")

    ident_bf = consts.tile([128, 128], BF, tag="idbf")
    make_identity(nc, ident_bf[:])
    ident_f32 = consts.tile([128, 128], F32, tag="idf32")
    make_identity(nc, ident_f32[:])
    eps_sb = consts.tile([128, 1], F32, tag="eps")
    nc.vector.memset(eps_sb[:], EPS)
    zero_sb = consts.tile([128, 1], F32, tag="zero")
    nc.vector.memset(zero_sb[:], 0.0)

    # persistent intermediates
    fused_sb = consts.tile([128, S, CCH, N], BF, tag="fused")
    lowT_sb = consts.tile([BOT, N], BF, tag="lowT")
    h_all = consts.tile([128, NCH, H], F32, tag="h_all")
    mv1 = consts.tile([128, NCH, 2], F32, tag="mv1")
    rstd1 = consts.tile([128, NCH], F32, tag="rstd1")
    hT_sb = consts.tile([128, CCH, N], BF, tag="hT")
    wh_all = consts.tile([128, NCH, HEADS * (D + 1)], BF, tag="wh")
    nc.vector.memset(
        wh_all[:].rearrange("p j (h x) -> p j h x", x=D + 1)[:, :, :, D], 1.0)
    sd_sb = consts.tile([128, NCH, 2 * HEADS], F32, tag="sd")
    srcb_sb = consts.tile([128, HEADS, N], BF, tag="srcb")
    hp_all = consts.tile([128, NCH, H], F32, tag="hp")
    mv2 = consts.tile([128, NCH, 2], F32, tag="mv2")
    rstd2 = consts.tile([128, NCH], F32, tag="rstd2")

    # ---------------- phase A: conv + silu ----------------
    ctxB = ExitStack()
    psA = ctxB.enter_context(tc.tile_pool(name="psB", bufs=2, space="PSUM"))
    psTr = ctxB.enter_context(tc.tile_pool(name="psTrB", bufs=2, space="PSUM"))
    ctxA = ExitStack()
    convp = ctxA.enter_context(tc.tile_pool(name="convp", bufs=3, space="PSUM"))
    lowp = ctxA.enter_context(tc.tile_pool(name="lowp", bufs=1, space="PSUM"))
    WINV = 1.0 / WSCALE
    for nch in range(2):               # halves of N, 512 wide
        if nch == 1:
            nc.scalar.dma_start(out=mask_sb[:], in_=mask_d[:])
        for i in range(S):
            for cout in range(CCH):
                ps = convp.tile([128, 512], F32, tag="conv")
                dil = 2 ** i
                for k in range(K):
                    # fp8 DoubleRow: one matmul contracts both channel
                    # chunks (lhsT [128,2,128], rhs [128,2,512])
                    sh = (k - 1) * dil
                    t0 = (i * K + k) * CCH
                    a = 8 + sh + nch * 512
                    nc.tensor.matmul(
                        ps[:],
                        lhsT=wt_sb[:, t0:t0 + 2, cout * 128:(cout + 1) * 128],
                        rhs=xpad[:, :, a:a + 512],
                        start=(k == 0), stop=(k == K - 1),
                        perf_mode=mybir.MatmulPerfMode.DoubleRow)
                dst = fused_sb[:, i, cout, nch * 512:nch * 512 + 512]
                bias_ap = bconv_sb[:, i * CCH + cout:i * CCH + cout + 1]
                if os.environ.get("BASS_SIM_COMPAT", "0") == "1":
                    # CoreSim has no Silu: sigmoid + fused (t+b)*sig on DVE
                    t_ = work.tile([128, 512], F32, tag="tconv")
                    nc.scalar.mul(t_[:], ps[:], WINV)
                    sg = work.tile([128, 512], F32, tag="sg")
                    nc.scalar.activation(
                        out=sg[:], in_=t_[:],
                        func=mybir.ActivationFunctionType.Sigmoid,
                        bias=bias_ap, scale=1.0)
                    nc.vector.scalar_tensor_tensor(
                        out=dst, in0=t_[:], scalar=bias_ap, in1=sg[:],
                        op0=mybir.AluOpType.add, op1=mybir.AluOpType.mult)
                else:
                    nc.scalar.activation(
                        out=dst, in_=ps[:],
                        func=mybir.ActivationFunctionType.Silu,
                        bias=bias_ap, scale=WINV)

        # phase A2 for this half: lowT = sum_i (a_i W_low)^T @ silu_i
        lps = lowp.tile([BOT, 512], F32, tag="low")
        first = True
        for i in range(S):
            for c in range(CCH):
                nc.tensor.matmul(
                    lps[:],
                    lhsT=wlow_sb[:, i * CCH + c, :],
                    rhs=fused_sb[:, i, c, nch * 512:nch * 512 + 512],
                    start=first, stop=(i == S - 1 and c == CCH - 1))
                first = False
        nc.vector.tensor_copy(out=lowT_sb[:, nch * 512:nch * 512 + 512],
                              in_=lps[:])

        # phase B for this half: high + residual + LN1 stats
        for q in range(nch * 4, nch * 4 + 4):
            hps = psA.tile([128, H], F32, tag="high")
            nc.tensor.matmul(hps[:], lhsT=lowT_sb[:, q * 128:(q + 1) * 128],
                             rhs=whigh_sb[:], start=True, stop=True)
            nc.vector.tensor_add(h_all[:, q, :], hps[:], xres_sb[:, q, :])
            st = statp.tile([128, 6], F32, tag="bn1")
            nc.vector.bn_stats(out=st[:], in_=h_all[:, q, :])
            nc.vector.bn_aggr(out=mv1[:, q, :], in_=st[:])

        # rstd1 per half = exp(-0.5 * ln(var + eps)); LN1-apply + transpose
        # for this half overlaps the other half's conv
        q0 = nch * 4
        nc.scalar.activation(out=rstd1[:, q0:q0 + 4], in_=mv1[:, q0:q0 + 4, 1],
                             func=mybir.ActivationFunctionType.Ln,
                             bias=eps_sb[:], scale=1.0)
        nc.scalar.activation(out=rstd1[:, q0:q0 + 4], in_=rstd1[:, q0:q0 + 4],
                             func=mybir.ActivationFunctionType.Exp,
                             bias=zero_sb[:], scale=-0.5)
        for q in range(q0, q0 + 4):
            hn = work.tile([128, H], BF, tag="hn")
            nc.vector.tensor_scalar(
                out=hn[:], in0=h_all[:, q, :],
                scalar1=mv1[:, q, 0:1], scalar2=rstd1[:, q:q + 1],
                op0=mybir.AluOpType.subtract, op1=mybir.AluOpType.mult)
            for c in range(CCH):
                tp = psTr.tile([128, 128], BF, tag="trh")
                nc.tensor.transpose(out=tp[:],
                                    in_=hn[:, c * 128:(c + 1) * 128],
                                    identity=ident_bf[:])
                nc.scalar.copy(out=hT_sb[:, c, q * 128:(q + 1) * 128],
                               in_=tp[:])
    ctxA.close()
    ctxB.close()
    # ---------------- phase C: GAT projections ----------------
    ctxC = ExitStack()
    psA = ctxC.enter_context(tc.tile_pool(name="psC", bufs=2, space="PSUM"))
    psTr = ctxC.enter_context(tc.tile_pool(name="psTrC", bufs=2, space="PSUM"))
    for j in range(NCH):
        gps = psA.tile([128, H + 2 * HEADS], F32, tag="gat")
        for c in range(CCH):
            nc.tensor.matmul(gps[:], lhsT=hT_sb[:, c, j * 128:(j + 1) * 128],
                             rhs=g_sb[:, c, :], start=(c == 0),
                             stop=(c == CCH - 1))
        whj = wh_all[:, j, :].rearrange("p (h x) -> p h x", x=D + 1)
        nc.scalar.copy(
            out=whj[:, :, 0:D],
            in_=gps[:, 0:H].rearrange("p (h x) -> p h x", x=D))
        nc.vector.tensor_copy(out=sd_sb[:, j, :], in_=gps[:, H:H + 2 * HEADS])

    # src_bcast[h][p, q] = src_h[q] for all p, via replicated-column matmul
    for h in range(HEADS):
        for half in range(2):
            sps = psTr.tile([128, 512], F32, tag="sbc")
            for c in range(CCH):
                nc.tensor.matmul(
                    sps[:], lhsT=wsr_sb[:, h, c, :],
                    rhs=hT_sb[:, c, half * 512:half * 512 + 512],
                    start=(c == 0), stop=(c == CCH - 1))
            nc.scalar.copy(out=srcb_sb[:, h, half * 512:half * 512 + 512],
                           in_=sps[:])

    ctxC.close()
    # ---------------- phase D: attention ----------------
    ctxD = ExitStack()
    attp = ctxD.enter_context(tc.tile_pool(name="attp", bufs=4, space="PSUM"))
    psTr = ctxD.enter_context(tc.tile_pool(name="psTrD", bufs=3, space="PSUM"))
    J_SCALAR = ()       # gpsimd add + ScalarE Prelu (empty: all chunks on DVE)
    for h in range(HEADS):
        ptl = ptlp.tile([128, NCH, N], BF, tag="ptl")
        pt = ptp.tile([128, NCH, N], BF, tag="pt")
        for j in J_SCALAR:
            cs = work.tile([128, N], BF, tag="cs")
            nc.gpsimd.tensor_tensor(out=cs[:], in0=srcb_sb[:, h, :],
                                    in1=mask_sb[:, j, :],
                                    op=mybir.AluOpType.add)
            nc.scalar.activation(
                out=ptl[:, j, :], in_=cs[:],
                func=mybir.ActivationFunctionType.Prelu,
                bias=sd_sb[:, j, HEADS + h:HEADS + h + 1],
                scale=1.0, alpha=LEAKY_SLOPE)
        for j in range(NCH):
            if j in J_SCALAR:
                continue
            # ptl = leaky(srcb[q] + dst[m] + mask) in one DVE pass
            nc.vector._custom_dve(
                LEAKY_OP, out=ptl[:, j, :], in0=srcb_sb[:, h, :],
                in1=mask_sb[:, j, :],
                s0=sd_sb[:, j, HEADS + h:HEADS + h + 1], imm2=LEAKY_SLOPE)
        # last head: quarter-split exp so downstream PE/DVE start sooner
        nexp = 4 if h == HEADS - 1 else 2
        step = NCH // nexp
        for part in range(nexp):
            nc.scalar.activation(
                out=pt[:, part * step:(part + 1) * step, :],
                in_=ptl[:, part * step:(part + 1) * step, :],
                func=mybir.ActivationFunctionType.Exp,
                bias=zero_sb[:], scale=1.0)

        hp0 = attp.tile([D + 1, 512], F32, tag="hpT")
        hp1 = attp.tile([D + 1, 512], F32, tag="hpT")
        for j in range(NCH):
            for half, hps_ in ((0, hp0), (1, hp1)):
                nc.tensor.matmul(
                    hps_[:],
                    lhsT=wh_all[:, j, h * (D + 1):(h + 1) * (D + 1)],
                    rhs=pt[:, j, half * 512:half * 512 + 512],
                    start=(j == 0), stop=(j == NCH - 1))
        hpt = work.tile([D + 1, N], F32, tag="hpt")
        nc.scalar.copy(out=hpt[:, 0:512], in_=hp0[:])
        nc.scalar.copy(out=hpt[:, 512:N], in_=hp1[:])
        for q in range(NCH):
            tq = psTr.tile([128, D + 1], F32, tag="trq")
            nc.tensor.transpose(out=tq[:], in_=hpt[:, q * 128:(q + 1) * 128],
                                identity=ident_f32[0:D + 1, 0:D + 1])
            rd = statp.tile([128, 1], F32, tag="rd")
            nc.vector.reciprocal(out=rd[:], in_=tq[:, D:D + 1])
            nc.vector.tensor_scalar_mul(
                out=hp_all[:, q, h * D:(h + 1) * D],
                in0=tq[:, 0:D], scalar1=rd[:])
            if h == HEADS - 1:
                # interleave LN2 stats with the last head's normalize so the
                # in-order DVE queue doesn't serialize them at the end
                st = statp.tile([128, 6], F32, tag="bn2")
                nc.vector.bn_stats(out=st[:], in_=hp_all[:, q, :])
                nc.vector.bn_aggr(out=mv2[:, q, :], in_=st[:])

    ctxD.close()
    # ---------------- phase E: ln2 + out ----------------
    nc.scalar.activation(out=rstd2[:], in_=mv2[:, :, 1],
                         func=mybir.ActivationFunctionType.Ln, bias=eps_sb[:],
                         scale=1.0)
    nc.scalar.activation(out=rstd2[:], in_=rstd2[:],
                         func=mybir.ActivationFunctionType.Exp, bias=zero_sb[:],
                         scale=-0.5)
    for q in range(NCH):
        ot = outp.tile([128, H], F16, tag="out")
        nc.vector.tensor_scalar(
            out=ot[:], in0=hp_all[:, q, :],
            scalar1=mv2[:, q, 0:1], scalar2=rstd2[:, q:q + 1],
            op0=mybir.AluOpType.subtract, op1=mybir.AluOpType.mult)
        eng = nc.sync if q % 2 == 0 else nc.scalar
        eng.dma_start(out=out_d[q * 128:(q + 1) * 128, :], in_=ot[:])


def _prep(inputs):
    """Host-side parameter folding. Returns per-core input maps."""
    bf16 = ml_dtypes.bfloat16
    f = lambda a: np.ascontiguousarray(np.asarray(a, np.float32))

    x = f(inputs["x"])
    adj = np.asarray(inputs["adj"])
    conv_w = f(inputs["conv_w"]); conv_b = f(inputs["conv_b"])
    bn_g = f(inputs["bn_g"]); bn_b = f(inputs["bn_b"])
    fw = f(inputs["fusion_weight"])
    W_low = f(inputs["W_low"]); b_low = f(inputs["b_low"])
    W_high = f(inputs["W_high"]); b_high = f(inputs["b_high"])
    ln1_g = f(inputs["ln1_g"]); ln1_b = f(inputs["ln1_b"])
    gat_W = f(inputs["gat_W"])
    a_src = f(inputs["a_src"]); a_dst = f(inputs["a_dst"])
    ln2_g = f(inputs["ln2_g"]); ln2_b = f(inputs["ln2_b"])

    trivial = dict(
        b_low=np.allclose(b_low, 0), b_high=np.allclose(b_high, 0),
        ln1=np.allclose(ln1_g, 1) and np.allclose(ln1_b, 0),
        ln2=np.allclose(ln2_g, 1) and np.allclose(ln2_b, 0))
    if not all(trivial.values()):
        raise NotImplementedError(f"non-trivial affine params: {trivial}")

    alpha = np.exp(fw - fw.max()); alpha /= alpha.sum()
    gprime = bn_g / np.float32(np.sqrt(1.0 + EPS))          # [S,H]
    bconv = conv_b * gprime + bn_b                           # [S,H]
    # Wt[i,k,cin,cout] = conv_w[i,cout,cin,k]*gprime[i,cout], pre-scaled by
    # WSCALE so e4m3 keeps ~3 mantissa bits (unscaled in the SiLU activation)
    Wt = np.transpose(conv_w, (0, 3, 2, 1)) * gprime[:, None, None, :] * WSCALE
    # [S,K,cin,H] -> [S*K*CCH,128,H] -> partition-major [128,S*K*CCH,H]
    Wt = Wt.reshape(S, K, CCH, 128, H).reshape(S * K * CCH, 128, H)
    Wt = np.ascontiguousarray(Wt.transpose(1, 0, 2))
    # bconv laid out [128, S*CCH]: column i*CCH+c holds channels c*128..c*128+127
    bconv_t = bconv.reshape(S, CCH, 128).transpose(2, 0, 1).reshape(128, S * CCH)

    WlowA = (alpha[:, None, None] * W_low[None]).reshape(S, CCH, 128, BOT)
    WlowA = np.ascontiguousarray(
        WlowA.reshape(S * CCH, 128, BOT).transpose(1, 0, 2))

    G = np.zeros((H, H + 2 * HEADS), np.float32)
    for h in range(HEADS):
        G[:, h * D:(h + 1) * D] = gat_W[h]
        G[:, H + h] = gat_W[h] @ a_src[h]
        G[:, H + HEADS + h] = gat_W[h] @ a_dst[h]
    Gr = np.ascontiguousarray(
        G.reshape(CCH, 128, H + 2 * HEADS).transpose(1, 0, 2))

    maskT = np.where(adj.T > 0, np.float32(0.0), np.float32(NEG))
    maskTr = np.ascontiguousarray(maskT.reshape(NCH, 128, N).transpose(1, 0, 2))

    # wsrcrep[:, h, c, j] = (gat_W[h] @ a_src[h])[c*128 + :]  (all 128 cols equal)
    wsrc = np.stack([gat_W[h] @ a_src[h] for h in range(HEADS)])  # [HEADS, H]
    wsrcrep = np.repeat(
        wsrc.reshape(HEADS, CCH, 128, 1), 128, axis=3).astype(np.float32)
    wsrcrep = np.ascontiguousarray(wsrcrep.transpose(2, 0, 1, 3))

    fp8 = ml_dtypes.float8_e4m3
    shared = {
        "wt": Wt.astype(fp8),
        "bconv": np.ascontiguousarray(bconv_t),
        "wlow": WlowA.astype(bf16),
        "whigh": W_high.astype(bf16),
        "gmat": Gr.astype(bf16),
        "maskT": maskTr.astype(bf16),
        "wsrcrep": wsrcrep.astype(bf16),
    }
    in_maps = []
    for b in range(B):
        xt = np.ascontiguousarray(x[b].T)                    # [H, N]
        m = dict(shared)
        m["xt"] = np.ascontiguousarray(
            xt.reshape(CCH, 128, N).transpose(1, 0, 2)).astype(fp8)
        m["xres"] = np.ascontiguousarray(
            x[b].reshape(NCH, 128, H).transpose(1, 0, 2)).astype(np.float16)
        in_maps.append(m)
    return in_maps, trivial


def kernel(**inputs) -> np.ndarray:
    in_maps, trivial = _prep(inputs)
    key = "k"
    if key not in _CACHED:
        _CACHED[key] = _build(trivial)
    nc = _CACHED[key]
    res = run_bass_kernel_spmd(nc, in_maps, list(range(B)))
    out = np.stack([res.results[i]["out"] for i in range(B)], axis=0)
    return out.astype(np.float32)


if __name__ == "__main__":
    import reference
    inputs = {k: np.asarray(v) for k, v in reference.setup_inputs().items()}
    got = kernel(**inputs)
    print("kernel output", got.shape, got.dtype)
